# revision 18
# baseline (speedup 1.0000x reference)
"""Trainium2 Bass kernel for nn_DGCNN6_homo (DGCNN with homophily features).

Strategy: shard the B=32 graphs across 8 NeuronCores (4 graphs/core, SPMD).
Per graph, each DynamicEdgeConv is computed as:
  - scores v_ij = x_i . x_j - s_j/2 via one augmented PE matmul (row of -1 /
    s/2 appended to the operands), so kNN ranking needs no elementwise pass.
    Feature tiles are fp16 so every big matmul runs at 1 cycle/row on the PE.
  - top-5 neighbor selection with a single DVE max8 + max_index pass over a
    [128, 2048] PSUM score tile (no split/merge stage)
  - neighbor features gathered with the SWDGE dma_gather (B_j = x_j W1b rows,
    token-major fp16), transposed back to feature-major on the PE while
    accumulating A_i = x_i (W1a - W1b) into the same PSUM tile
  - per-edge 2-layer MLP on PE/ACT (leaky relu via Lrelu activation),
    neighbor aggregation as packed-fp16 DVE adds (4x DVE mode)
Homophily (equal-value fraction over the k=50 kNN graph) is exactly zero
unless a graph has duplicate coordinate values; the rare duplicates are
counted exactly on the host (a few rows of distance computation at most) and
enter the device's final MLP as 4 scalars per graph.
"""
import numpy as np

N = 2048          # nodes per graph
B_TOTAL = 32      # graphs
G = 4             # graphs per core
NB = 16           # 128-row blocks per graph
KEC = 5           # edge-conv k
NCORES = 8

_MODULE = None


# --------------------------------------------------------------------------
# device module
# --------------------------------------------------------------------------
def _build_module():
    import os
    _LIM = int(os.environ.get("K_LIM", "99"))
    _GLIM = int(os.environ.get("K_GLIM", "99"))
    CB = int(os.environ.get("K_CB", "2"))
    NCH = NB // CB
    WPAD = 64 if CB > 6 else 32
    import concourse.bacc as bacc
    import concourse.mybir as mybir
    from concourse.tile import TileContext
    from concourse.masks import make_identity

    fp32 = mybir.dt.float32
    f16 = mybir.dt.float16
    i16 = mybir.dt.int16
    u16 = mybir.dt.uint16
    AF = mybir.ActivationFunctionType
    ALU = mybir.AluOpType
    AX = mybir.AxisListType

    nc = bacc.Bacc(None, target_bir_lowering=False, debug=False)

    def win(name, shape, dt=fp32):
        return nc.dram_tensor(name, shape, dt, kind="ExternalInput")

    xx_in = win("xx", [G * N, 4])
    c1w1d = win("c1w1d", [4, 64], f16);  c1w1b = win("c1w1b", [4, 128], f16)
    c1b1 = win("c1b1", [64, 1]);  c1w2 = win("c1w2", [64, 64], f16)
    c1b2 = win("c1b2", [64, 1])
    c2w1d = win("c2w1d", [64, 128], f16);  c2w1b = win("c2w1b", [64, 128], f16)
    c2b1 = win("c2b1", [128, 1]);  c2w2 = win("c2w2", [128, 64], f16)
    c2b2 = win("c2b2", [64, 1])
    l1x1 = win("l1x1", [64, 512], f16);  l1x2 = win("l1x2", [64, 512], f16)
    l1x3 = win("l1x3", [68, 512], f16)
    l1b1 = win("l1b1", [128, 4])
    l1w2 = win("l1w2", [128, 4, 256], f16);  l1b2 = win("l1b2", [128, 2])
    mw1 = win("mw1", [128, 3, 256]);  mb1 = win("mb1", [128, 2])
    mw2 = win("mw2", [128, 2, 3]);  mb2 = win("mb2", [3, 1])
    hvals = win("hvals", [4, G])
    repeye = win("repeye", [16, 128])

    outT = nc.dram_tensor("outT", [3, G], fp32, kind="ExternalOutput")

    dB1 = [nc.dram_tensor(f"dB1_{g}", [N, 128], f16) for g in range(G)]
    dB2 = [nc.dram_tensor(f"dB2_{g}", [N, 128], f16) for g in range(G)]
    dB3 = [nc.dram_tensor(f"dB3_{g}", [N, 128], f16) for g in range(G)]

    with TileContext(nc) as tc:
        import contextlib
        ctx = contextlib.ExitStack()
        with ctx:
            cst = ctx.enter_context(tc.tile_pool(name="cst", bufs=1))
            pg = ctx.enter_context(tc.tile_pool(name="pg", bufs=1))
            paugR = ctx.enter_context(tc.tile_pool(name="paugR", bufs=2))
            pstage = ctx.enter_context(tc.tile_pool(name="pstage", bufs=2))
            patab = ctx.enter_context(tc.tile_pool(name="patab", bufs=2))
            pgath = ctx.enter_context(tc.tile_pool(name="pgath", bufs=2))
            pidx = ctx.enter_context(tc.tile_pool(name="pidx", bufs=3))
            ph = ctx.enter_context(tc.tile_pool(name="ph", bufs=6))
            psm = ctx.enter_context(tc.tile_pool(name="psm", bufs=3))
            po1 = ctx.enter_context(tc.tile_pool(name="po1", bufs=2))
            psel = ctx.enter_context(tc.tile_pool(name="psel", bufs=2))
            pbig = ctx.enter_context(tc.tile_pool(name="pbig", bufs=1, space="PSUM"))
            pmlp = ctx.enter_context(tc.tile_pool(name="pmlp", bufs=2, space="PSUM"))
            pw = pmlp

            # ---------------- constants / weights
            ident = cst.tile([128, 128], fp32)
            make_identity(nc, ident[:])
            identh = cst.tile([128, 128], f16)
            nc.vector.tensor_copy(identh[:], ident[:])
            half4 = cst.tile([4, 1], f16)
            nc.vector.memset(half4[:], 0.5)
            half64 = cst.tile([64, 1], f16)
            nc.vector.memset(half64[:], 0.5)

            def wload(dram, shape, dt=fp32):
                t = cst.tile(shape, dt, tag=f"w_{dram.name}")
                nc.sync.dma_start(t[:], dram[:])
                return t

            c1w1d_t = wload(c1w1d, [4, 64], f16);  c1w1b_t = wload(c1w1b, [4, 128], f16)
            c1b1_t = wload(c1b1, [64, 1]);  c1w2_t = wload(c1w2, [64, 64], f16)
            c1b2_t = wload(c1b2, [64, 1])
            c2w1d_t = wload(c2w1d, [64, 128], f16);  c2w1b_t = wload(c2w1b, [64, 128], f16)
            c2b1_t = wload(c2b1, [128, 1]);  c2w2_t = wload(c2w2, [128, 64], f16)
            c2b2_t = wload(c2b2, [64, 1])
            l1x1_t = wload(l1x1, [64, 512], f16);  l1x2_t = wload(l1x2, [64, 512], f16)
            l1x3_t = wload(l1x3, [68, 512], f16);  l1b1_t = wload(l1b1, [128, 4])
            l1w2_t = wload(l1w2, [128, 4, 256], f16);  l1b2_t = wload(l1b2, [128, 2])
            mw1_t = wload(mw1, [128, 3, 256]);  mb1_t = wload(mb1, [128, 2])
            mw2_t = wload(mw2, [128, 2, 3]);  mb2_t = wload(mb2, [3, 1])

            repeye_t = wload(repeye, [16, 128])
            neg1 = cst.tile([1, N], f16)
            nc.vector.memset(neg1[:], -1.0)
            T0 = cst.tile([128, 3, G], fp32)
            nc.vector.memset(T0[:], 0.0)
            nc.sync.dma_start(T0[0:4, 2, :], hvals[:])

            # ------------------------------------------------------------------
            def build_srow(augR, xrows, hones, nfeat, augrow, via_dma=False):
                """augR[augrow, :] = sum_f xrows[f, :]^2 / 2."""
                sq = psm.tile([128, N], f16, tag="sq")
                nc.scalar.activation(sq[0:nfeat, :], xrows, AF.Square)
                sp = pbig.tile([1, 2048], fp32, space="PSUM", tag="big")
                for c in range(4):
                    nc.tensor.matmul(sp[:, 512 * c:512 * (c + 1)],
                                     lhsT=hones,
                                     rhs=sq[0:nfeat, 512 * c:512 * (c + 1)],
                                     start=True, stop=True)
                if via_dma:
                    # engine writes must start at a 32-aligned partition;
                    # stage at partition 0 and DMA into the target row
                    stg = psm.tile([1, N], f16, tag="srowstg")
                    nc.scalar.activation(stg[:], sp[:], AF.Copy)
                    nc.sync.dma_start(augR[augrow:augrow + 1, :], stg[:])
                else:
                    nc.scalar.activation(augR[augrow:augrow + 1, :], sp[:], AF.Copy)

            def build_atab(w1d_t, xrows, H):
                """A^T [H, N] = w1d^T @ x^T  (feature-major, fp16)."""
                at = patab.tile([H, N], f16, tag="atab")
                ap = pbig.tile([H, 2048], fp32, space="PSUM", tag="big")
                for c in range(4):
                    nc.tensor.matmul(ap[:, 512 * c:512 * (c + 1)],
                                     lhsT=w1d_t,
                                     rhs=xrows[:, 512 * c:512 * (c + 1)],
                                     start=True, stop=True)
                nc.scalar.activation(at[:, 0:1024], ap[:, 0:1024], AF.Copy)
                nc.scalar.activation(at[:, 1024:2048], ap[:, 1024:2048], AF.Copy)
                return at

            def build_btab(w1b_t, xrows, H, dB):
                """dB [N, H] token-major fp16 = x @ W1b, written via PSUM->DRAM.
                Batched 4 node-blocks per PSUM tile to shorten the chain."""
                gb = 512 // H          # blocks per psum bank
                for q in range(NB // gb):
                    bp = pw.tile([128, gb, H], fp32, space="PSUM", tag="mlp")
                    for b in range(gb):
                        nc.tensor.matmul(
                            bp[:, b, :],
                            lhsT=xrows[:, 128 * (gb * q + b):128 * (gb * q + b + 1)],
                            rhs=w1b_t,
                            start=True, stop=True)
                    stg = psm.tile([128, gb, H], f16, tag="bstg")
                    nc.scalar.activation(stg[:], bp[:], AF.Copy)
                    nc.sync.dma_start(
                        dB[128 * gb * q:128 * gb * (q + 1), :].rearrange(
                            "(b p) h -> p b h", b=gb),
                        stg[:])

            def conv(g, ci, augL, augR, at_t, dB, w2t, b1t, b2t, H, out_rows, HG=None):
                HG = HG or H
                # software pipeline: selection/gather of chunk c+1 is emitted
                # before the MLP of chunk c so the DVE never stalls behind it
                gaths = [None] * NCH

                def sel_block(bl, c4, idx8f):
                    b = CB * c4 + bl
                    vp = pbig.tile([128, 2048], fp32, space="PSUM", tag="big")
                    for c in range(4):
                        nc.tensor.matmul(vp[:, 512 * c:512 * (c + 1)],
                                         lhsT=augL[:, 128 * b:128 * (b + 1)],
                                         rhs=augR[:, 512 * c:512 * (c + 1)],
                                         start=True, stop=True)
                    # stage scores to SBUF so the PSUM tile frees early and the
                    # PE can compute the next block's scores during selection
                    sc = psel.tile([128, 2048], fp32, tag="sc")
                    nc.scalar.activation(sc[:, 0:1024], vp[:, 0:1024], AF.Copy)
                    nc.scalar.activation(sc[:, 1024:2048], vp[:, 1024:2048],
                                         AF.Copy)
                    vals8 = psm.tile([128, 8], fp32, tag="vals8")
                    nc.vector.max(vals8[:], sc[:])
                    iA = psm.tile([128, 8], u16, tag="idxA")
                    nc.vector.max_index(iA[:], vals8[:], sc[:])
                    nc.vector.tensor_copy(idx8f[:, bl, :], iA[:, 0:KEC])

                def idx_gather(c4, idx8f):
                    wps = pmlp.tile([16, 8, WPAD], fp32, space="PSUM", tag="mlp")
                    for phh in range(8):
                        nc.tensor.matmul(wps[:, phh, 0:CB * KEC],
                                         lhsT=ident[:, 16 * phh:16 * (phh + 1)],
                                         rhs=idx8f[:],
                                         start=True, stop=True)
                    wsb = psm.tile([16, CB * 40], fp32, tag="wsb")
                    nc.scalar.activation(
                        wsb[:], wps[:, :, 0:CB * KEC].rearrange("q p e -> q e p"),
                        AF.Copy)
                    rps = pmlp.tile([128, CB * 40], fp32, space="PSUM", tag="mlp")
                    for rc in range(0, CB * 40, 512):
                        re = min(rc + 512, CB * 40)
                        nc.tensor.matmul(rps[:, rc:re], lhsT=repeye_t[:],
                                         rhs=wsb[:, rc:re], start=True, stop=True)
                    widx = pidx.tile([128, CB * 40], i16, tag="widx")
                    nc.vector.tensor_copy(widx[:], rps[:])
                    gath = pgath.tile([128, 1, CB * KEC * 128], f16, tag="gath")
                    nc.gpsimd.dma_gather(
                        out_ap=gath[:], in_ap=dB[:], idxs_ap=widx[:],
                        num_idxs=CB * 128 * KEC, num_idxs_reg=CB * 128 * KEC,
                        elem_size=HG, transpose=True, single_packet=False,
                    )
                    gaths[c4] = gath

                ysbs = [None] * NCH

                def mlp_tile(c4, tl):
                    gath = gaths[c4]
                    if ysbs[c4] is None:
                        ysbs[c4] = [None] * CB
                    t = CB * c4 + tl
                    zps = pmlp.tile([H, 640], fp32, space="PSUM", tag="mlp")
                    nd = at_t[:, 128 * t:128 * (t + 1)]
                    nc.tensor.matmul(
                        zps[:, 0:512].rearrange("h (s n) -> h s n", s=4),
                        lhsT=identh[0:H, 0:H],
                        rhs=nd.unsqueeze(1).to_broadcast([H, 4, 128]),
                        start=True, stop=False)
                    nc.tensor.matmul(zps[:, 512:640], lhsT=identh[0:H, 0:H],
                                     rhs=nd, start=True, stop=False)
                    bt = gath[0:H, 0, 640 * tl:640 * (tl + 1)]
                    nc.tensor.matmul(zps[:, 0:512], lhsT=identh[0:H, 0:H],
                                     rhs=bt[:, 0:512], start=False, stop=True)
                    nc.tensor.matmul(zps[:, 512:640], lhsT=identh[0:H, 0:H],
                                     rhs=bt[:, 512:640], start=False, stop=True)
                    ht = ph.tile([H, 640], f16, tag="h")
                    nc.scalar.activation(ht[:], zps[:], AF.Lrelu,
                                         bias=b1t[:], scale=1.0, alpha=0.01)
                    yps = pw.tile([64, 640], fp32, space="PSUM", tag="mlp")
                    nc.tensor.matmul(yps[:, 0:512], lhsT=w2t[:],
                                     rhs=ht[:, 0:512], start=True, stop=True)
                    nc.tensor.matmul(yps[:, 512:640], lhsT=w2t[:],
                                     rhs=ht[:, 512:640], start=True, stop=True)
                    ysb = ph.tile([64, 640], f16, tag="ysb")
                    nc.scalar.activation(ysb[:], yps[:], AF.Lrelu,
                                         bias=b2t[:], scale=1.0, alpha=0.01)
                    ysbs[c4][tl] = ysb

                def mlp_reduce(c4):
                    for tl in range(CB):
                        t = CB * c4 + tl
                        y = ysbs[c4][tl]
                        s01 = psm.tile([64, 128], f16, tag="s01")
                        s23 = psm.tile([64, 128], f16, tag="s23")
                        nc.gpsimd.tensor_tensor(s01[:], y[:, 0:128],
                                                y[:, 128:256], op=ALU.add)
                        nc.gpsimd.tensor_tensor(s23[:], y[:, 256:384],
                                                y[:, 384:512], op=ALU.add)
                        nc.gpsimd.tensor_tensor(s01[:], s01[:], s23[:],
                                                op=ALU.add)
                        nc.gpsimd.tensor_tensor(
                            out_rows[:, 128 * t:128 * (t + 1)],
                            s01[:], y[:, 512:640], op=ALU.add)

                for c4 in range(NCH):
                    idx8f = psm.tile([128, CB, KEC], fp32, tag="idx8f")
                    for bl in range(CB):
                        sel_block(bl, c4, idx8f)
                    if c4 >= 1:
                        for bl in range(CB):
                            mlp_tile(c4 - 1, bl)
                    idx_gather(c4, idx8f)
                    if c4 >= 2:
                        mlp_reduce(c4 - 2)
                for tl in range(CB):
                    mlp_tile(NCH - 1, tl)
                mlp_reduce(NCH - 2)
                mlp_reduce(NCH - 1)

            # ------------------------------------------------------------------
            for g in range(min(G, _GLIM)):
                # ---------- prep: xx -> feature-major fp16 + aug rows
                xx_tok = psm.tile([128, NB, 4], fp32, tag="xx_tok")
                nc.sync.dma_start(
                    xx_tok[:],
                    xx_in.rearrange("(gg b p) f -> gg p b f", gg=G, p=128)[g],
                )
                augL1 = pstage.tile([5, N], f16, tag="augL1")
                augR1 = paugR.tile([5, N], f16, tag="augR")
                xpa = pbig.tile([4, 2048], fp32, space="PSUM", tag="big")
                for b in range(NB):
                    nc.tensor.matmul(xpa[:, 128 * b:128 * (b + 1)],
                                     lhsT=xx_tok[:, b, :], rhs=ident[:],
                                     is_transpose=True,
                                     start=(b % 4 == 0), stop=(b % 4 == 3))
                nc.scalar.activation(augL1[0:4, 0:1024], xpa[:, 0:1024], AF.Copy)
                nc.scalar.activation(augL1[0:4, 1024:2048], xpa[:, 1024:2048],
                                     AF.Copy)
                nc.vector.tensor_copy(augR1[0:4, :], augL1[0:4, :])
                nc.sync.dma_start(augL1[4:5, :], neg1[:])
                build_srow(augR1, augL1[0:4, :], half4[:], 4, 4, via_dma=True)

                if _LIM < 1:
                    continue
                a1t = build_atab(c1w1d_t[:], augL1[0:4, :], 64)
                build_btab(c1w1b_t[:], augL1[0:4, :], 128, dB1[g])

                if _LIM < 2:
                    continue
                augL2 = pg.tile([65, N], f16, tag="augL2")
                conv(g, 0, augL1[:], augR1[:], a1t, dB1[g],
                     c1w2_t, c1b1_t, c1b2_t, 64, augL2[0:64, :], HG=128)

                if _LIM < 3:
                    continue
                # ---------- conv2 prep
                nc.gpsimd.memset(augL2[64:65, :], -1.0)
                augR2 = paugR.tile([65, N], f16, tag="augR")
                nc.vector.tensor_copy(augR2[0:64, :], augL2[0:64, :])
                build_srow(augR2, augL2[0:64, :], half64[:], 64, 64)
                a2t = build_atab(c2w1d_t[:], augL2[0:64, :], 128)
                build_btab(c2w1b_t[:], augL2[0:64, :], 128, dB2[g])

                if _LIM < 4:
                    continue
                augL3 = pg.tile([65, N], f16, tag="augL3")
                conv(g, 1, augL2[:], augR2[:], a2t, dB2[g],
                     c2w2_t, c2b1_t, c2b2_t, 128, augL3[0:64, :])

                if _LIM < 5:
                    continue
                # ---------- conv3 prep (shared conv2 weights)
                nc.gpsimd.memset(augL3[64:65, :], -1.0)
                augR3 = paugR.tile([65, N], f16, tag="augR")
                nc.vector.tensor_copy(augR3[0:64, :], augL3[0:64, :])
                build_srow(augR3, augL3[0:64, :], half64[:], 64, 64)
                a3t = build_atab(c2w1d_t[:], augL3[0:64, :], 128)
                build_btab(c2w1b_t[:], augL3[0:64, :], 128, dB3[g])

                if _LIM < 6:
                    continue
                x3xx = pg.tile([68, N], f16, tag="x3xx")
                conv(g, 2, augL3[:], augR3[:], a3t, dB3[g],
                     c2w2_t, c2b1_t, c2b2_t, 128, x3xx[0:64, :])
                nc.vector.tensor_copy(x3xx[64:68, :], augL1[0:4, :])

                if _LIM < 7:
                    continue
                # ---------- lin1 (feat = [x1 | x2 | x3 | xx], K-chunks) + pool
                red = psm.tile([128, 8], fp32, tag="red")
                for q in range(4):
                    o1q = po1.tile([128, 4, 512], f16, tag="o1q")
                    sl = slice(512 * q, 512 * (q + 1))
                    for m in range(4):
                        ops = pmlp.tile([128, 512], fp32, space="PSUM", tag="mlp")
                        nc.tensor.matmul(ops[:], lhsT=l1x1_t[:, 128 * m:128 * (m + 1)],
                                         rhs=augL2[0:64, sl], start=True, stop=False)
                        nc.tensor.matmul(ops[:], lhsT=l1x2_t[:, 128 * m:128 * (m + 1)],
                                         rhs=augL3[0:64, sl], start=False, stop=False)
                        nc.tensor.matmul(ops[:], lhsT=l1x3_t[:, 128 * m:128 * (m + 1)],
                                         rhs=x3xx[:, sl], start=False, stop=True)
                        nc.scalar.activation(o1q[:, m, :], ops[:], AF.Lrelu,
                                             bias=l1b1_t[:, m:m + 1], scale=1.0,
                                             alpha=0.01)
                    for mo in range(2):
                        o2ps = pw.tile([128, 512], fp32, space="PSUM", tag="mlp")
                        for k in range(4):
                            nc.tensor.matmul(
                                o2ps[:],
                                lhsT=l1w2_t[:, k, 128 * mo:128 * (mo + 1)],
                                rhs=o1q[:, k, :],
                                start=(k == 0), stop=(k == 3))
                        nc.vector.tensor_reduce(red[:, 4 * mo + q:4 * mo + q + 1],
                                                o2ps[:], axis=AX.X, op=ALU.add)
                for mo in range(2):
                    psum = psm.tile([128, 1], fp32, tag="psum1")
                    nc.vector.tensor_reduce(psum[:], red[:, 4 * mo:4 * mo + 4],
                                            axis=AX.X, op=ALU.add)
                    nc.vector.tensor_scalar(
                        T0[:, mo, g:g + 1], psum[:], 1.0 / N,
                        l1b2_t[:, mo:mo + 1],
                        op0=ALU.mult, op1=ALU.add)

            # ---------- final MLP over all 4 graphs
            T0L = cst.tile([128, 3, G], fp32)
            nc.scalar.activation(T0L[:], T0[:], AF.Lrelu, alpha=0.01)
            h1 = cst.tile([128, 2, G], fp32)
            for mo in range(2):
                zp = pmlp.tile([128, G], fp32, space="PSUM", tag="mlp")
                for k in range(3):
                    nc.tensor.matmul(zp[:],
                                     lhsT=mw1_t[:, k, 128 * mo:128 * (mo + 1)],
                                     rhs=T0L[:, k, :],
                                     start=(k == 0), stop=(k == 2))
                nc.scalar.activation(h1[:, mo, :], zp[:], AF.Lrelu,
                                     bias=mb1_t[:, mo:mo + 1], scale=1.0,
                                     alpha=0.01)
            op = pw.tile([3, G], fp32, space="PSUM", tag="mlp")
            for k in range(2):
                nc.tensor.matmul(op[:], lhsT=mw2_t[:, k, :], rhs=h1[:, k, :],
                                 start=(k == 0), stop=(k == 1))
            osb = cst.tile([3, G], fp32)
            nc.scalar.activation(osb[:], op[:], AF.Identity, bias=mb2_t[:])
            nc.sync.dma_start(outT[:], osb[:])

    nc.compile()
    return nc


def _get_module():
    global _MODULE
    if _MODULE is None:
        _MODULE = _build_module()
    return _MODULE


# --------------------------------------------------------------------------
# host: exact homophily (duplicate values only; usually all-zero)
# --------------------------------------------------------------------------
def _homophily_host(xx):
    xx = xx.reshape(B_TOTAL, N, 4)
    h = np.zeros((B_TOTAL, 4), np.float32)
    for b in range(B_TOTAL):
        xg = xx[b].astype(np.float32)
        s = None
        for c in range(4):
            vals = xg[:, c]
            u, inv, cnt = np.unique(vals, return_inverse=True,
                                    return_counts=True)
            if (cnt <= 1).all():
                continue
            if s is None:
                s = (xg.astype(np.float32) ** 2).sum(axis=1)
            for ui in np.where(cnt > 1)[0]:
                nodes = np.where(inv == ui)[0]
                for i in nodes:
                    d = s + s[i] - 2.0 * (xg @ xg[i])
                    d = d.astype(np.float32)
                    d[i] = d[i] + np.float32(1e9)
                    for j in nodes:
                        if j == i:
                            continue
                        rank = int((d < d[j]).sum()) + int(
                            ((d == d[j]) & (np.arange(N) < j)).sum())
                        if rank < 50:
                            h[b, c] += 1.0
    return h / np.float32(N * 50.0)


# --------------------------------------------------------------------------
# host entry point
# --------------------------------------------------------------------------
def _prepare_in_maps(inputs):
    x = np.ascontiguousarray(np.asarray(inputs["x"], np.float32))
    pos = np.ascontiguousarray(np.asarray(inputs["pos"], np.float32))
    w = {k: np.ascontiguousarray(np.asarray(inputs[k], np.float32)) for k in
         ("c1w1", "c1b1", "c1w2", "c1b2", "c2w1", "c2b1", "c2w2", "c2b2",
          "l1w1", "l1b1", "l1w2", "l1b2", "mw1", "mb1", "mw2", "mb2")}

    xx = np.concatenate([x, pos], axis=1)                      # [B*N, 4]
    hv = _homophily_host(xx)                                   # [32, 4]

    def f16(a):
        return np.ascontiguousarray(np.asarray(a, np.float16))

    # conv weight decompositions
    c1w1 = w["c1w1"]
    c1w1d = f16(c1w1[0:4] - c1w1[4:8])
    c1w1b = np.zeros((4, 128), np.float16)
    c1w1b[:, 0:64] = f16(c1w1[4:8])
    c1w1b = np.ascontiguousarray(c1w1b)
    c2w1 = w["c2w1"]
    c2w1d = f16(c2w1[0:64] - c2w1[64:128])
    c2w1b = f16(c2w1[64:128])

    # lin1 rows reordered to [x1, x2, x3, xx]
    l1w1 = w["l1w1"]                                           # [196, 512]
    l1x1 = f16(l1w1[4:68])
    l1x2 = f16(l1w1[68:132])
    l1x3 = f16(np.concatenate([l1w1[132:196], l1w1[0:4]], axis=0))
    l1b1c = np.ascontiguousarray(w["l1b1"].reshape(4, 128).T)  # [128, 4]
    l1w2c = f16(w["l1w2"].reshape(4, 128, 256).transpose(1, 0, 2))
    l1b2c = np.ascontiguousarray(w["l1b2"].reshape(2, 128).T)  # [128, 2]
    mw1p = np.zeros((384, 256), np.float32)
    mw1p[0:260] = w["mw1"]
    mw1c = np.ascontiguousarray(mw1p.reshape(3, 128, 256).transpose(1, 0, 2))
    mb1c = np.ascontiguousarray(w["mb1"].reshape(2, 128).T)    # [128, 2]
    mw2c = np.ascontiguousarray(w["mw2"].reshape(2, 128, 3).transpose(1, 0, 2))
    mb2c = np.ascontiguousarray(w["mb2"].reshape(3, 1))

    shared = dict(
        c1w1d=c1w1d, c1w1b=c1w1b, c1b1=w["c1b1"].reshape(64, 1),
        c1w2=f16(w["c1w2"]), c1b2=w["c1b2"].reshape(64, 1),
        c2w1d=c2w1d, c2w1b=c2w1b, c2b1=w["c2b1"].reshape(128, 1),
        c2w2=f16(w["c2w2"]), c2b2=w["c2b2"].reshape(64, 1),
        l1x1=l1x1, l1x2=l1x2, l1x3=l1x3, l1b1=l1b1c,
        l1w2=l1w2c, l1b2=l1b2c,
        mw1=mw1c, mb1=mb1c, mw2=mw2c, mb2=mb2c,
        repeye=np.ascontiguousarray(np.tile(np.eye(16, dtype=np.float32), 8)),
    )

    in_maps = []
    for c in range(NCORES):
        im = dict(shared)
        im["xx"] = np.ascontiguousarray(xx[G * N * c:G * N * (c + 1)])
        im["hvals"] = np.ascontiguousarray(hv[G * c:G * (c + 1)].T)  # [4ch, G]
        in_maps.append(im)
    return in_maps


def kernel(**inputs):
    in_maps = _prepare_in_maps(inputs)
    nc = _get_module()
    from concourse.bass_utils import run_bass_kernel_spmd
    res = run_bass_kernel_spmd(nc, in_maps, list(range(NCORES)))
    out = np.concatenate([r["outT"].T for r in res.results], axis=0)  # [32, 3]
    return np.ascontiguousarray(out.astype(np.float32))
